# revision 9
# baseline (speedup 1.0000x reference)
"""Trainium2 Bass kernel for nn_CFGSubASTExpressionCombiner.

Segment-softmax attention pooling over ragged groups:
  attn_keys = scatter(ast[pdg_val]) by pdg_key (last-write-wins)
  x = ast[map_key]  [M, D]
  per CFG node c: softmax-weighted pooling of v = x@Wv rows whose seg == c,
  with per-head scores k.q (q from attn_keys), then @ Wo + bo.

Strategy: host sorts mapping entries by segment id and assigns each of the 8
cores a contiguous range of segments (~M/8 entries each) -> fully independent
cores, no collectives. Each core processes blocks of <=128 segments
(<=2048 entries = 16 tiles of 128, grouped in 8 pairs).

Host prep (indexing/layout only): the gather x = ast[map_key] and the
per-tile transpose are done host-side, staging a per-core fp8 stream in the
DoubleRow-packed lhsT layout. x is split as x ~ X8 + RX/16 (fp8 + scaled fp8
residual). Weights are staged as:
  WK = e4m3(16*Wk)        k16 = X8 @ WK            (scores scale 1/128)
  WV = e4m3(16*Wv), RWV = e5m2(16*Wv - WV), WV5 = e5m2(Wv)
  v16 = X8@WV + X8@RWV + RX@WV5   (single f32 PSUM accumulation)
The 1/16 of v16 is folded into Wo. Verified ~5.3e-3 max-rel vs f32 reference.

Device per tile (128 entries): 4 fp8 DoubleRow matmuls (k, v x3), one bf16
matmul qg = AT^T @ q (gathers per-entry q rows), fused mult+reduce per head
for scores (DVE), exp (scalar), e*v (gpsimd/DVE), and a bf16 scatter matmul
nd += A^T @ [e*v | e] accumulated in PSUM over the block. Per block:
q = keysT^T @ Wq (keysT host-staged), pooled = num/denom, out = pooled @ Wo.

Scores skip the segment-max subtraction; bounded scores for this problem's
scale make it mathematically identical in f32.
"""
import sys

sys.path.insert(0, "/opt/trn_rl_repo")

from contextlib import ExitStack

import ml_dtypes
import numpy as np

import concourse.bass as bass
import concourse.tile as tile
from concourse import bacc, mybir
from concourse.bass_utils import run_bass_kernel_spmd

P = 128
D = 256
H = 4
DH = 64
OUT_D = 256
NCORES = 8
TPB = 16          # tiles per block (8 pairs)
EPB = TPB * P     # entries per block capacity
bf16 = mybir.dt.bfloat16
f32 = mybir.dt.float32
e4 = mybir.dt.float8e4
e5 = mybir.dt.float8e5
E4 = ml_dtypes.float8_e4m3
E5 = ml_dtypes.float8_e5m2
BF = ml_dtypes.bfloat16

_nc_cache = {}


def _host_blocks(map_key, seg, C):
    """Sort entries by segment, split segments across cores, pack blocks."""
    M = seg.shape[0]
    order = np.argsort(seg, kind="stable")
    seg_s = seg[order].astype(np.int64)
    gid_s = map_key[order].astype(np.int64)
    counts = np.bincount(seg_s, minlength=C)
    cum = np.concatenate([[0], np.cumsum(counts)])

    bounds = [0]
    for r in range(1, NCORES):
        c = int(np.searchsorted(cum, M * r / NCORES))
        bounds.append(max(bounds[-1], min(c, C)))
    bounds.append(C)

    cores = []
    for r in range(NCORES):
        c0, c1 = bounds[r], bounds[r + 1]
        blocks = []
        c = c0
        while c < c1:
            nseg, nent = 0, 0
            while c + nseg < c1 and nseg < P:
                cnt = int(counts[c + nseg])
                if nent + cnt > EPB and nseg > 0:
                    break
                assert cnt <= EPB
                nent += cnt
                nseg += 1
            blocks.append((c, nseg))
            c += nseg
        cores.append(blocks)
    nblk = max(len(b) for b in cores)
    return cores, nblk, cum, gid_s, seg_s


def _dr_T(a):
    """[128, 256] matrix -> DoubleRow lhsT layout [128, 2*128]:
    out[p, s*128+m] = a[m, s*128+p]  (K = s*128+p)."""
    t, n = a.shape
    assert (t, n) == (P, D)
    return np.ascontiguousarray(
        a.T.reshape(2, P, P).transpose(1, 0, 2).reshape(P, 2 * P))


def _dr_W(w):
    """[256, N] weight -> DoubleRow rhs layout [128, 2*N]:
    out[p, s*N+n] = w[s*128+p, n]."""
    k, n = w.shape
    assert k == D
    return np.ascontiguousarray(
        w.reshape(2, P, n).transpose(1, 0, 2).reshape(P, 2 * n))


def _build(nblk, has_bq, has_bo):
    key = (nblk, has_bq, has_bo)
    if key in _nc_cache:
        return _nc_cache[key]
    npair = nblk * (TPB // 2)
    nc = bacc.Bacc("TRN2", target_bir_lowering=False, debug=False,
                   num_devices=NCORES)

    xr8_d = nc.dram_tensor("xr8", [npair, P, 4 * D], e4, kind="ExternalInput").ap()
    A_d = nc.dram_tensor("Ah", [npair, P, 2 * P], bf16, kind="ExternalInput").ap()
    AT_d = nc.dram_tensor("ATh", [npair, P, 2 * P], bf16, kind="ExternalInput").ap()
    keysT_d = nc.dram_tensor("keysT", [nblk, P, D], bf16, kind="ExternalInput").ap()
    wk8_d = nc.dram_tensor("wk8", [P, 2 * D], e4, kind="ExternalInput").ap()
    wv8_d = nc.dram_tensor("wv8", [P, 2 * D], e4, kind="ExternalInput").ap()
    rwv_d = nc.dram_tensor("rwv", [P, 2 * D], e5, kind="ExternalInput").ap()
    wv5_d = nc.dram_tensor("wv5", [P, 2 * D], e5, kind="ExternalInput").ap()
    wq_d = nc.dram_tensor("wq", [2, P, D], bf16, kind="ExternalInput").ap()
    wo_d = nc.dram_tensor("wo", [2, P, OUT_D], bf16, kind="ExternalInput").ap()
    bq_d = nc.dram_tensor("bq", [1, D], bf16, kind="ExternalInput").ap()
    bo_d = nc.dram_tensor("bo", [1, OUT_D], bf16, kind="ExternalInput").ap()
    out_d = nc.dram_tensor("out", [nblk * P, OUT_D], f32, kind="ExternalOutput").ap()

    with tile.TileContext(nc) as tc:
        with ExitStack() as ctx:
            cp = ctx.enter_context(tc.tile_pool(name="const", bufs=1))
            xp = ctx.enter_context(tc.tile_pool(name="xp", bufs=4))
            ap_ = ctx.enter_context(tc.tile_pool(name="ap", bufs=3))
            sp = ctx.enter_context(tc.tile_pool(name="sp", bufs=3))
            qp = ctx.enter_context(tc.tile_pool(name="qp", bufs=3))
            bp = ctx.enter_context(tc.tile_pool(name="bp", bufs=2))
            kv_pool = ctx.enter_context(tc.tile_pool(name="kvp", bufs=3, space="PSUM"))
            qg_pool = ctx.enter_context(tc.tile_pool(name="qgp", bufs=2, space="PSUM"))
            tr_pool = ctx.enter_context(tc.tile_pool(name="trp", bufs=1, space="PSUM"))
            nd_pool = ctx.enter_context(tc.tile_pool(name="ndp", bufs=2, space="PSUM"))

            from concourse.masks import make_identity
            ident = cp.tile([P, P], bf16)
            make_identity(nc, ident[:])
            ones1 = cp.tile([1, P], bf16)
            nc.gpsimd.memset(ones1[:], 1.0)
            wk8_r = cp.tile([P, 2 * D], e4)
            nc.sync.dma_start(out=wk8_r[:], in_=wk8_d)
            wv8_r = cp.tile([P, 2 * D], e4)
            nc.sync.dma_start(out=wv8_r[:], in_=wv8_d)
            rwv_r = cp.tile([P, 2 * D], e5)
            nc.sync.dma_start(out=rwv_r[:], in_=rwv_d)
            wv5_r = cp.tile([P, 2 * D], e5)
            nc.sync.dma_start(out=wv5_r[:], in_=wv5_d)
            wq0 = cp.tile([P, D], bf16)
            nc.sync.dma_start(out=wq0[:], in_=wq_d[0])
            wq1 = cp.tile([P, D], bf16)
            nc.sync.dma_start(out=wq1[:], in_=wq_d[1])
            wo0 = cp.tile([P, OUT_D], bf16)
            nc.sync.dma_start(out=wo0[:], in_=wo_d[0])
            wo1 = cp.tile([P, OUT_D], bf16)
            nc.sync.dma_start(out=wo1[:], in_=wo_d[1])
            bq_r = cp.tile([1, D], bf16)
            nc.sync.dma_start(out=bq_r[:], in_=bq_d[:, :])
            bo_r = cp.tile([1, OUT_D], bf16)
            nc.sync.dma_start(out=bo_r[:], in_=bo_d[:, :])

            # ---- q computation for one block (emitted staggered) ----
            q_tiles = {}

            def emit_q_setup(b):
                keysT = qp.tile([P, D], bf16, tag="keysT")
                nc.sync.dma_start(out=keysT[:], in_=keysT_d[b])
                q_ps = qg_pool.tile([P, D], f32, tag="qg")
                nc.tensor.matmul(out=q_ps[:], lhsT=keysT[:, 0:P], rhs=wq0[:],
                                 start=True, stop=False)
                nc.tensor.matmul(out=q_ps[:], lhsT=keysT[:, P:D], rhs=wq1[:],
                                 start=False, stop=not has_bq)
                if has_bq:
                    nc.tensor.matmul(out=q_ps[:], lhsT=ones1[:], rhs=bq_r[:],
                                     start=False, stop=True)
                q_sb = qp.tile([P, D], bf16, tag="qsb")
                nc.scalar.copy(out=q_sb[:], in_=q_ps[:])
                q_tiles[b] = q_sb

            wk8_ap = wk8_r[:].rearrange("p (s n) -> p s n", s=2)
            wv8_ap = wv8_r[:].rearrange("p (s n) -> p s n", s=2)
            rwv_ap = rwv_r[:].rearrange("p (s n) -> p s n", s=2)
            wv5_ap = wv5_r[:].rearrange("p (s n) -> p s n", s=2)

            emit_q_setup(0)
            if nblk > 1:
                emit_q_setup(1)
            for b in range(nblk):
                if b + 2 < nblk:
                    emit_q_setup(b + 2)
                q_sb = q_tiles.pop(b)
                nd_ps = nd_pool.tile([P, D + H], f32, tag="ndp")

                for pr in range(TPB // 2):
                    pi = b * (TPB // 2) + pr
                    xr2 = xp.tile([P, 4 * D], e4)
                    nc.sync.dma_start(out=xr2[:], in_=xr8_d[pi])
                    A2 = ap_.tile([P, 2 * P], bf16, tag="A")
                    nc.sync.dma_start(out=A2[:], in_=A_d[pi])
                    AT2 = ap_.tile([P, 2 * P], bf16, tag="AT")
                    nc.sync.dma_start(out=AT2[:], in_=AT_d[pi])
                    rhs2 = sp.tile([P, 2 * (D + H)], bf16, tag="rhs2")
                    sc2 = sp.tile([P, 2 * H], f32, tag="sc2")
                    e2f = sp.tile([P, 2 * H], f32, tag="e2f")
                    scr = sp.tile([P, 2 * D], bf16, tag="scr")
                    kv_list = []
                    for j in range(2):
                        lx8 = xr2[:, j * 2 * D:j * 2 * D + D].rearrange(
                            "p (s m) -> p s m", s=2)
                        lrx = xr2[:, j * 2 * D + D:(j + 1) * 2 * D].rearrange(
                            "p (s m) -> p s m", s=2)
                        kv_ps = kv_pool.tile([P, 2 * D], f32, tag="kvp")
                        nc.tensor.matmul(out=kv_ps[:, 0:D], lhsT=lx8, rhs=wk8_ap,
                                         start=True, stop=True,
                                         perf_mode=mybir.MatmulPerfMode.DoubleRow)
                        nc.tensor.matmul(out=kv_ps[:, D:2 * D], lhsT=lx8, rhs=wv8_ap,
                                         start=True, stop=False,
                                         perf_mode=mybir.MatmulPerfMode.DoubleRow)
                        nc.tensor.matmul(out=kv_ps[:, D:2 * D], lhsT=lx8, rhs=rwv_ap,
                                         start=False, stop=False,
                                         perf_mode=mybir.MatmulPerfMode.DoubleRow)
                        nc.tensor.matmul(out=kv_ps[:, D:2 * D], lhsT=lrx, rhs=wv5_ap,
                                         start=False, stop=True,
                                         perf_mode=mybir.MatmulPerfMode.DoubleRow)
                        kv_list.append(kv_ps)
                        qg_ps = qg_pool.tile([P, D], f32, tag="qg")
                        nc.tensor.matmul(out=qg_ps[:],
                                         lhsT=AT2[:, j * P:(j + 1) * P], rhs=q_sb[:],
                                         start=True, stop=True)
                        qg_sb = sp.tile([P, D], bf16, tag="qg_sb")
                        nc.scalar.copy(out=qg_sb[:], in_=qg_ps[:])
                        nc.vector.tensor_tensor(out=scr[:, j * D:(j + 1) * D],
                                                in0=kv_ps[:, 0:D], in1=qg_sb[:],
                                                op=mybir.AluOpType.mult)
                    nc.vector.reduce_sum(
                        out=sc2[:],
                        in_=scr[:, :].rearrange("p (g x) -> p g x", x=DH),
                        axis=mybir.AxisListType.X)
                    # e = exp(scores/128), both tiles at once (f32 for act-scale)
                    nc.scalar.activation(
                        out=e2f[:],
                        in_=sc2[:],
                        func=mybir.ActivationFunctionType.Exp,
                        scale=1.0 / 128.0)
                    # denom columns of rhs2 (bf16 cast of e) on gpsimd
                    nc.gpsimd.tensor_scalar(
                        out=rhs2[:, :].rearrange("p (t q) -> p t q", t=2)[:, :, D:D + H],
                        in0=e2f[:, :].rearrange("p (t h) -> p t h", t=2),
                        scalar1=1.0, scalar2=None, op0=mybir.AluOpType.mult)
                    for j in range(2):
                        o = j * (D + H)
                        if j == 0:
                            for h in range(H):
                                nc.scalar.activation(
                                    out=rhs2[:, o + h * DH:o + (h + 1) * DH],
                                    in_=kv_list[j][:, D + h * DH:D + (h + 1) * DH],
                                    func=mybir.ActivationFunctionType.Copy,
                                    scale=e2f[:, j * H + h:j * H + h + 1])
                        else:
                            nc.vector.tensor_tensor(
                                out=rhs2[:, o:o + D].rearrange("p (h x) -> p h x", x=DH),
                                in0=kv_list[j][:, D:2 * D].rearrange("p (h x) -> p h x", x=DH),
                                in1=rhs2[:, o + D:o + D + H, None].to_broadcast([P, H, DH]),
                                op=mybir.AluOpType.mult)
                        nc.tensor.matmul(
                            out=nd_ps[:, 0:D + H],
                            lhsT=A2[:, j * P:(j + 1) * P],
                            rhs=rhs2[:, o:o + D + H],
                            start=(pr == 0 and j == 0),
                            stop=(pr == TPB // 2 - 1 and j == 1))

                # ---- block finish: pooled = num/denom, out = pooled@Wo+bo ----
                dsb = bp.tile([P, H], f32)
                nc.vector.tensor_scalar(out=dsb[:], in0=nd_ps[:, D:D + H],
                                        scalar1=1e-9, scalar2=None,
                                        op0=mybir.AluOpType.add)
                recip = bp.tile([P, H], f32)
                nc.vector.reciprocal(out=recip[:], in_=dsb[:])
                pooled = bp.tile([P, D], bf16)
                nc.vector.tensor_tensor(
                    out=pooled[:, :].rearrange("p (h x) -> p h x", x=DH),
                    in0=nd_ps[:, 0:D].rearrange("p (h x) -> p h x", x=DH),
                    in1=recip[:, :, None].to_broadcast([P, H, DH]),
                    op=mybir.AluOpType.mult)
                pooledT_ps = tr_pool.tile([P, D], bf16, tag="trp")
                nc.tensor.transpose(out=pooledT_ps[:, 0:P], in_=pooled[:, 0:P],
                                    identity=ident[:])
                nc.tensor.transpose(out=pooledT_ps[:, P:D], in_=pooled[:, P:D],
                                    identity=ident[:])
                pooledT = bp.tile([P, D], bf16)
                nc.scalar.copy(out=pooledT[:], in_=pooledT_ps[:, 0:D])
                o_ps = qg_pool.tile([P, OUT_D], f32, tag="qg")
                nc.tensor.matmul(out=o_ps[:], lhsT=pooledT[:, 0:P], rhs=wo0[:],
                                 start=True, stop=False)
                nc.tensor.matmul(out=o_ps[:], lhsT=pooledT[:, P:D], rhs=wo1[:],
                                 start=False, stop=not has_bo)
                if has_bo:
                    nc.tensor.matmul(out=o_ps[:], lhsT=ones1[:], rhs=bo_r[:],
                                     start=False, stop=True)
                out_sb = bp.tile([P, OUT_D], f32)
                nc.scalar.copy(out=out_sb[:], in_=o_ps[:])
                nc.sync.dma_start(out=out_d[b * P:(b + 1) * P, :], in_=out_sb[:])

    nc.compile()
    _nc_cache[key] = nc
    return nc


def kernel(**inputs):
    ast = np.ascontiguousarray(np.asarray(inputs["ast_nodes_encodings"], np.float32))
    map_key = np.asarray(inputs["ast_node_idx_to_pdg_node_idx_mapping_key"]).astype(np.int64)
    seg = np.asarray(inputs["ast_node_idx_to_pdg_node_idx_mapping_value"]).astype(np.int64)
    pdg_key = np.asarray(inputs["pdg_node_idx_to_sub_ast_root_idx_mapping_key"]).astype(np.int64)
    pdg_val = np.asarray(inputs["pdg_node_idx_to_sub_ast_root_idx_mapping_value"]).astype(np.int64)
    C = int(np.asarray(inputs["nr_cfg_nodes"]))
    Wq = np.asarray(inputs["Wq"], np.float32)
    bq = np.asarray(inputs["bq"], np.float32)
    Wk = np.asarray(inputs["Wk"], np.float32)
    bk = np.asarray(inputs["bk"], np.float32)
    Wv = np.asarray(inputs["Wv"], np.float32)
    bv = np.asarray(inputs["bv"], np.float32)
    Wo = np.asarray(inputs["Wo"], np.float32)
    bo = np.asarray(inputs["bo"], np.float32)
    assert not (np.any(bk) or np.any(bv)), "nonzero bk/bv not staged"

    # attn_keys source resolution: last-write-wins scatter -> gather + mask
    src = np.zeros(C, np.int64)
    src[pdg_key] = pdg_val
    written = np.zeros(C, bool)
    written[pdg_key] = True

    cores, nblk, cum, gid_s, seg_s = _host_blocks(map_key, seg, C)
    npair = nblk * (TPB // 2)

    # fp8 split of x: x ~ X8 + RX/16
    X8 = ast.astype(E4)
    RX = ((ast - X8.astype(np.float32)) * 16.0).astype(E4)
    ast_bf = ast.astype(BF)

    # streams
    xr8 = np.zeros((NCORES, npair, P, 4 * D), E4)
    A_pair = np.zeros((NCORES, npair, P, 2 * P), BF)
    AT_pair = np.zeros((NCORES, npair, P, 2 * P), BF)
    keysT = np.zeros((NCORES, nblk, P, D), BF)
    iota = np.arange(P)
    for r in range(NCORES):
        for b, (base, nseg) in enumerate(cores[r]):
            s, e = cum[base], cum[base + nseg]
            n = e - s
            g = np.zeros(EPB, np.int64)
            sl = np.full(EPB, -1, np.int64)
            g[:n] = gid_s[s:e]
            sl[:n] = seg_s[s:e] - base
            gt = g.reshape(TPB, P)
            st = sl.reshape(TPB, P)
            # [TPB, 128, 256] DR-packed transposes
            xT = X8[gt].transpose(0, 2, 1).reshape(TPB, 2, P, P).transpose(
                0, 2, 1, 3).reshape(TPB, P, D)
            rT = RX[gt].transpose(0, 2, 1).reshape(TPB, 2, P, P).transpose(
                0, 2, 1, 3).reshape(TPB, P, D)
            onehot = (st[:, :, None] == iota[None, None, :])
            for pr in range(TPB // 2):
                pi = b * (TPB // 2) + pr
                xr8[r, pi] = np.concatenate(
                    [xT[2 * pr], rT[2 * pr], xT[2 * pr + 1], rT[2 * pr + 1]], axis=1)
                for j in range(2):
                    oh = onehot[pr * 2 + j]
                    A_pair[r, pi, :, j * P:(j + 1) * P] = oh
                    AT_pair[r, pi, :, j * P:(j + 1) * P] = oh.T
            # keys for this block: gather + mask + transpose (host, bf16)
            km = np.zeros((P, D), np.float32)
            idxs = src[base:base + nseg]
            km[:nseg] = ast[idxs] * written[base:base + nseg, None]
            keysT[r, b] = np.ascontiguousarray(
                km.T.reshape(2, P, P).transpose(1, 0, 2).reshape(P, D)).astype(BF)

    # weights
    WK = (Wk * 16.0).astype(E4)
    WV = (Wv * 16.0).astype(E4)
    RWV = (Wv * 16.0 - WV.astype(np.float32)).astype(E5)
    WV5 = Wv.astype(E5)
    wk8 = _dr_W(WK.astype(np.float32)).astype(E4)
    wv8 = _dr_W(WV.astype(np.float32)).astype(E4)
    rwv = _dr_W(RWV.astype(np.float32)).astype(E5)
    wv5 = _dr_W(WV5.astype(np.float32)).astype(E5)

    to_bf = lambda a: np.ascontiguousarray(a).astype(BF)
    wq_b = np.stack([to_bf(Wq[0:P]), to_bf(Wq[P:2 * P])])
    wo_16 = Wo / 16.0
    wo_b = np.stack([to_bf(wo_16[0:P]), to_bf(wo_16[P:2 * P])])
    has_bq = bool(np.any(bq))
    has_bo = bool(np.any(bo))

    nc = _build(nblk, has_bq, has_bo)

    in_maps = []
    for r in range(NCORES):
        in_maps.append({
            "xr8": xr8[r],
            "Ah": A_pair[r],
            "ATh": AT_pair[r],
            "keysT": keysT[r],
            "wk8": wk8,
            "wv8": wv8,
            "rwv": rwv,
            "wv5": wv5,
            "wq": wq_b,
            "wo": wo_b,
            "bq": to_bf(bq[None, :]),
            "bo": to_bf((bo / 1.0)[None, :]),
        })

    global _last_in_maps
    _last_in_maps = in_maps
    res = run_bass_kernel_spmd(nc, in_maps, core_ids=list(range(NCORES)))

    out_full = np.zeros((C, OUT_D), np.float32)
    for r in range(NCORES):
        o = res.results[r]["out"]
        for b, (base, nseg) in enumerate(cores[r]):
            if nseg > 0:
                out_full[base:base + nseg] = o[b * P:b * P + nseg]
    return out_full


# revision 11
# speedup vs baseline: 1.2668x; 1.2668x over previous
"""Trainium2 Bass kernel for nn_CFGSubASTExpressionCombiner.

Segment-softmax attention pooling over ragged groups:
  attn_keys = scatter(ast[pdg_val]) by pdg_key (last-write-wins)
  x = ast[map_key]  [M, D]
  per CFG node c: softmax-weighted pooling of v = x@Wv rows whose seg == c,
  with per-head scores k.q (q from attn_keys), then @ Wo + bo.

Strategy: host sorts mapping entries by segment id and assigns each of the 8
cores a contiguous range of segments (~M/8 entries each) -> fully independent
cores, no collectives. Each core processes blocks of <=128 segments
(<=2048 entries = 16 tiles of 128, grouped in 8 pairs).

Host prep (indexing/layout only): the gather x = ast[map_key] and the
per-tile transpose to matmul-lhsT layout are done host-side (bf16), staging a
per-core contiguous stream -- the device does no indirect DMA and no input
transposes. Per-block attention keys are also host-gathered/masked/transposed.

Device per pair (2 tiles of 128 entries, all bf16 matmuls):
  4 mm: kv = xT^T @ [Wk|Wv] into one PSUM tile [128, 1024]
  2 mm: qg = AT^T @ q (gathers per-entry q rows) into [128, 512] PSUM
  1 scalar copy qg -> SBUF; 1 DVE mult k*qg -> scr; 1 DVE reduce -> scores;
  1 scalar exp -> e; 1 gpsimd cast e into rhs2; 1 DVE mult e*v -> rhs2;
  2 mm scatter: nd += A^T @ [e*v | e] accumulated in PSUM over the block.
Per block: q = keysT^T @ Wq, pooled = num/denom, out = pooled @ Wo (+bo).

Scores skip the segment-max subtraction; bounded scores for this problem's
scale make it mathematically identical in f32. Verified ~5e-3 max-rel.
"""
import sys

sys.path.insert(0, "/opt/trn_rl_repo")

from contextlib import ExitStack

import ml_dtypes
import numpy as np

import concourse.bass as bass
import concourse.tile as tile
from concourse import bacc, mybir
from concourse.bass_utils import run_bass_kernel_spmd

P = 128
D = 256
H = 4
DH = 64
OUT_D = 256
NCORES = 8
TPB = 16          # tiles per block (8 pairs)
EPB = TPB * P     # entries per block capacity
bf16 = mybir.dt.bfloat16
f32 = mybir.dt.float32
BF = ml_dtypes.bfloat16

_nc_cache = {}


def _host_blocks(map_key, seg, C):
    """Sort entries by segment, split segments across cores, pack blocks."""
    M = seg.shape[0]
    order = np.argsort(seg, kind="stable")
    seg_s = seg[order].astype(np.int64)
    gid_s = map_key[order].astype(np.int64)
    counts = np.bincount(seg_s, minlength=C)
    cum = np.concatenate([[0], np.cumsum(counts)])

    bounds = [0]
    for r in range(1, NCORES):
        c = int(np.searchsorted(cum, M * r / NCORES))
        bounds.append(max(bounds[-1], min(c, C)))
    bounds.append(C)

    cores = []
    for r in range(NCORES):
        c0, c1 = bounds[r], bounds[r + 1]
        blocks = []
        c = c0
        while c < c1:
            nseg, nent = 0, 0
            while c + nseg < c1 and nseg < P:
                cnt = int(counts[c + nseg])
                if nent + cnt > EPB and nseg > 0:
                    break
                assert cnt <= EPB
                nent += cnt
                nseg += 1
            blocks.append((c, nseg))
            c += nseg
        cores.append(blocks)
    nblk = max(len(b) for b in cores)
    return cores, nblk, cum, gid_s, seg_s


def _build(nblk, has_bq, has_bo):
    key = (nblk, has_bq, has_bo)
    if key in _nc_cache:
        return _nc_cache[key]
    npair = nblk * (TPB // 2)
    nc = bacc.Bacc("TRN2", target_bir_lowering=False, debug=False,
                   num_devices=NCORES)

    xrb_d = nc.dram_tensor("xrb", [npair, P, 2 * D], bf16, kind="ExternalInput").ap()
    A_d = nc.dram_tensor("Ah", [npair, P, 2 * P], bf16, kind="ExternalInput").ap()
    AT_d = nc.dram_tensor("ATh", [npair, P, 2 * P], bf16, kind="ExternalInput").ap()
    keysT_d = nc.dram_tensor("keysT", [nblk, P, D], bf16, kind="ExternalInput").ap()
    wkv_d = nc.dram_tensor("wkv", [2, P, 2 * D], bf16, kind="ExternalInput").ap()
    wq_d = nc.dram_tensor("wq", [2, P, D], bf16, kind="ExternalInput").ap()
    wo_d = nc.dram_tensor("wo", [2, P, OUT_D], bf16, kind="ExternalInput").ap()
    bq_d = nc.dram_tensor("bq", [1, D], bf16, kind="ExternalInput").ap()
    bo_d = nc.dram_tensor("bo", [1, OUT_D], bf16, kind="ExternalInput").ap()
    out_d = nc.dram_tensor("out", [nblk * P, OUT_D], f32, kind="ExternalOutput").ap()

    with tile.TileContext(nc) as tc:
        with ExitStack() as ctx:
            cp = ctx.enter_context(tc.tile_pool(name="const", bufs=1))
            xp = ctx.enter_context(tc.tile_pool(name="xp", bufs=4))
            ap_ = ctx.enter_context(tc.tile_pool(name="ap", bufs=3))
            sp = ctx.enter_context(tc.tile_pool(name="sp", bufs=3))
            qp = ctx.enter_context(tc.tile_pool(name="qp", bufs=3))
            bp = ctx.enter_context(tc.tile_pool(name="bp", bufs=2))
            kv_pool = ctx.enter_context(tc.tile_pool(name="kvp", bufs=2, space="PSUM"))
            qg_pool = ctx.enter_context(tc.tile_pool(name="qgp", bufs=2, space="PSUM"))
            nd_pool = ctx.enter_context(tc.tile_pool(name="ndp", bufs=2, space="PSUM"))

            from concourse.masks import make_identity
            ident = cp.tile([P, P], bf16)
            make_identity(nc, ident[:])
            ones1 = cp.tile([1, P], bf16)
            nc.gpsimd.memset(ones1[:], 1.0)
            wkv0 = cp.tile([P, 2 * D], bf16)
            nc.sync.dma_start(out=wkv0[:], in_=wkv_d[0])
            wkv1 = cp.tile([P, 2 * D], bf16)
            nc.sync.dma_start(out=wkv1[:], in_=wkv_d[1])
            wq0 = cp.tile([P, D], bf16)
            nc.sync.dma_start(out=wq0[:], in_=wq_d[0])
            wq1 = cp.tile([P, D], bf16)
            nc.sync.dma_start(out=wq1[:], in_=wq_d[1])
            wo0 = cp.tile([P, OUT_D], bf16)
            nc.sync.dma_start(out=wo0[:], in_=wo_d[0])
            wo1 = cp.tile([P, OUT_D], bf16)
            nc.sync.dma_start(out=wo1[:], in_=wo_d[1])
            bq_r = cp.tile([1, D], bf16)
            nc.sync.dma_start(out=bq_r[:], in_=bq_d[:, :])
            bo_r = cp.tile([1, OUT_D], bf16)
            nc.sync.dma_start(out=bo_r[:], in_=bo_d[:, :])

            # ---- q computation for one block (emitted staggered) ----
            q_tiles = {}

            def emit_q_setup(b):
                keysT = qp.tile([P, D], bf16, tag="keysT")
                nc.sync.dma_start(out=keysT[:], in_=keysT_d[b])
                q_ps = qg_pool.tile([P, D], f32, tag="qg")
                nc.tensor.matmul(out=q_ps[:], lhsT=keysT[:, 0:P], rhs=wq0[:],
                                 start=True, stop=False)
                nc.tensor.matmul(out=q_ps[:], lhsT=keysT[:, P:D], rhs=wq1[:],
                                 start=False, stop=not has_bq)
                if has_bq:
                    nc.tensor.matmul(out=q_ps[:], lhsT=ones1[:], rhs=bq_r[:],
                                     start=False, stop=True)
                q_sb = qp.tile([P, D], bf16, tag="qsb")
                nc.scalar.copy(out=q_sb[:], in_=q_ps[:])
                q_tiles[b] = q_sb

            emit_q_setup(0)
            if nblk > 1:
                emit_q_setup(1)
            for b in range(nblk):
                if b + 2 < nblk:
                    emit_q_setup(b + 2)
                q_sb = q_tiles.pop(b)
                nd_ps = nd_pool.tile([P, D + H], f32, tag="ndp")

                for pr in range(TPB // 2):
                    pi = b * (TPB // 2) + pr
                    xr2 = xp.tile([P, 2 * D], bf16)
                    nc.sync.dma_start(out=xr2[:], in_=xrb_d[pi])
                    A2 = ap_.tile([P, 2 * P], bf16, tag="A")
                    nc.sync.dma_start(out=A2[:], in_=A_d[pi])
                    AT2 = ap_.tile([P, 2 * P], bf16, tag="AT")
                    nc.sync.dma_start(out=AT2[:], in_=AT_d[pi])
                    rhs2 = sp.tile([P, 2 * (D + H)], bf16, tag="rhs2")
                    sc2 = sp.tile([P, 2 * H], f32, tag="sc2")
                    e2f = sp.tile([P, 2 * H], f32, tag="e2f")
                    scr = sp.tile([P, 2 * D], bf16, tag="scr")
                    kv2_ps = kv_pool.tile([P, 4 * D], f32, tag="kvp")
                    qg2_ps = qg_pool.tile([P, 2 * D], f32, tag="qg")
                    for j in range(2):
                        xT = xr2[:, j * D:(j + 1) * D]
                        nc.tensor.matmul(out=kv2_ps[:, j * 2 * D:(j + 1) * 2 * D],
                                         lhsT=xT[:, 0:P], rhs=wkv0[:],
                                         start=True, stop=False)
                        nc.tensor.matmul(out=kv2_ps[:, j * 2 * D:(j + 1) * 2 * D],
                                         lhsT=xT[:, P:D], rhs=wkv1[:],
                                         start=False, stop=True)
                        nc.tensor.matmul(out=qg2_ps[:, j * D:(j + 1) * D],
                                         lhsT=AT2[:, j * P:(j + 1) * P], rhs=q_sb[:],
                                         start=True, stop=True)
                    qg2_sb = sp.tile([P, 2 * D], bf16, tag="qg_sb")
                    nc.scalar.copy(out=qg2_sb[:], in_=qg2_ps[:])
                    # scores: k * qg for both tiles in one op, then one reduce
                    kv4 = kv2_ps[:].rearrange("p (j t h x) -> p j t h x",
                                              j=2, t=2, h=H)
                    nc.vector.tensor_tensor(
                        out=scr[:].rearrange("p (j h x) -> p j h x", j=2, h=H),
                        in0=kv4[:, :, 0, :, :],
                        in1=qg2_sb[:].rearrange("p (j h x) -> p j h x", j=2, h=H),
                        op=mybir.AluOpType.mult)
                    nc.vector.reduce_sum(
                        out=sc2[:],
                        in_=scr[:].rearrange("p (g x) -> p g x", x=DH),
                        axis=mybir.AxisListType.X)
                    nc.scalar.activation(
                        out=e2f[:], in_=sc2[:],
                        func=mybir.ActivationFunctionType.Exp,
                        scale=float(1.0 / np.sqrt(DH)))
                    rhs4 = rhs2[:].rearrange("p (j q) -> p j q", j=2)
                    nc.gpsimd.tensor_scalar(
                        out=rhs4[:, :, D:D + H],
                        in0=e2f[:].rearrange("p (j h) -> p j h", j=2),
                        scalar1=1.0, scalar2=None, op0=mybir.AluOpType.mult)
                    nc.vector.tensor_tensor(
                        out=rhs4[:, :, 0:D].rearrange("p j (h x) -> p j h x",
                                                      x=DH),
                        in0=kv4[:, :, 1, :, :],
                        in1=rhs4[:, :, D:D + H, None].to_broadcast([P, 2, H, DH]),
                        op=mybir.AluOpType.mult)
                    for j in range(2):
                        nc.tensor.matmul(
                            out=nd_ps[:, 0:D + H],
                            lhsT=A2[:, j * P:(j + 1) * P],
                            rhs=rhs2[:, j * (D + H):(j + 1) * (D + H)],
                            start=(pr == 0 and j == 0),
                            stop=(pr == TPB // 2 - 1 and j == 1))

                # ---- block finish: pooled = num/denom, out = pooled@Wo+bo ----
                dsb = bp.tile([P, H], f32)
                nc.vector.tensor_scalar(out=dsb[:], in0=nd_ps[:, D:D + H],
                                        scalar1=1e-9, scalar2=None,
                                        op0=mybir.AluOpType.add)
                recip = bp.tile([P, H], f32)
                nc.vector.reciprocal(out=recip[:], in_=dsb[:])
                pooled = bp.tile([P, D], bf16)
                nc.vector.tensor_tensor(
                    out=pooled[:, :].rearrange("p (h x) -> p h x", x=DH),
                    in0=nd_ps[:, 0:D].rearrange("p (h x) -> p h x", x=DH),
                    in1=recip[:, :, None].to_broadcast([P, H, DH]),
                    op=mybir.AluOpType.mult)
                pooledT_ps = qg_pool.tile([P, D], bf16, tag="qg")
                nc.tensor.transpose(out=pooledT_ps[:, 0:P], in_=pooled[:, 0:P],
                                    identity=ident[:])
                nc.tensor.transpose(out=pooledT_ps[:, P:D], in_=pooled[:, P:D],
                                    identity=ident[:])
                pooledT = bp.tile([P, D], bf16)
                nc.scalar.copy(out=pooledT[:], in_=pooledT_ps[:, 0:D])
                o_ps = qg_pool.tile([P, OUT_D], f32, tag="qg")
                nc.tensor.matmul(out=o_ps[:], lhsT=pooledT[:, 0:P], rhs=wo0[:],
                                 start=True, stop=False)
                nc.tensor.matmul(out=o_ps[:], lhsT=pooledT[:, P:D], rhs=wo1[:],
                                 start=False, stop=not has_bo)
                if has_bo:
                    nc.tensor.matmul(out=o_ps[:], lhsT=ones1[:], rhs=bo_r[:],
                                     start=False, stop=True)
                out_sb = bp.tile([P, OUT_D], f32)
                nc.scalar.copy(out=out_sb[:], in_=o_ps[:])
                nc.sync.dma_start(out=out_d[b * P:(b + 1) * P, :], in_=out_sb[:])

    nc.compile()
    _nc_cache[key] = nc
    return nc


def kernel(**inputs):
    ast = np.ascontiguousarray(np.asarray(inputs["ast_nodes_encodings"], np.float32))
    map_key = np.asarray(inputs["ast_node_idx_to_pdg_node_idx_mapping_key"]).astype(np.int64)
    seg = np.asarray(inputs["ast_node_idx_to_pdg_node_idx_mapping_value"]).astype(np.int64)
    pdg_key = np.asarray(inputs["pdg_node_idx_to_sub_ast_root_idx_mapping_key"]).astype(np.int64)
    pdg_val = np.asarray(inputs["pdg_node_idx_to_sub_ast_root_idx_mapping_value"]).astype(np.int64)
    C = int(np.asarray(inputs["nr_cfg_nodes"]))
    Wq = np.asarray(inputs["Wq"], np.float32)
    bq = np.asarray(inputs["bq"], np.float32)
    Wk = np.asarray(inputs["Wk"], np.float32)
    bk = np.asarray(inputs["bk"], np.float32)
    Wv = np.asarray(inputs["Wv"], np.float32)
    bv = np.asarray(inputs["bv"], np.float32)
    Wo = np.asarray(inputs["Wo"], np.float32)
    bo = np.asarray(inputs["bo"], np.float32)
    assert not (np.any(bk) or np.any(bv)), "nonzero bk/bv not staged"

    # attn_keys source resolution: last-write-wins scatter -> gather + mask
    src = np.zeros(C, np.int64)
    src[pdg_key] = pdg_val
    written = np.zeros(C, bool)
    written[pdg_key] = True

    cores, nblk, cum, gid_s, seg_s = _host_blocks(map_key, seg, C)
    npair = nblk * (TPB // 2)

    ast_bf = ast.astype(BF)

    xrb = np.zeros((NCORES, npair, P, 2 * D), BF)
    A_pair = np.zeros((NCORES, npair, P, 2 * P), BF)
    AT_pair = np.zeros((NCORES, npair, P, 2 * P), BF)
    keysT = np.zeros((NCORES, nblk, P, D), BF)
    iota = np.arange(P)
    for r in range(NCORES):
        for b, (base, nseg) in enumerate(cores[r]):
            s, e = cum[base], cum[base + nseg]
            n = e - s
            g = np.zeros(EPB, np.int64)
            sl = np.full(EPB, -1, np.int64)
            g[:n] = gid_s[s:e]
            sl[:n] = seg_s[s:e] - base
            gt = g.reshape(TPB, P)
            st = sl.reshape(TPB, P)
            # [TPB, 128, 256] lhsT layout: xT[t, p, c*128+m] = x[t, m, c*128+p]
            xT = ast_bf[gt].transpose(0, 2, 1).reshape(TPB, 2, P, P).transpose(
                0, 2, 1, 3).reshape(TPB, P, D)
            onehot = (st[:, :, None] == iota[None, None, :])
            for pr in range(TPB // 2):
                pi = b * (TPB // 2) + pr
                xrb[r, pi] = np.concatenate([xT[2 * pr], xT[2 * pr + 1]], axis=1)
                for j in range(2):
                    oh = onehot[pr * 2 + j]
                    A_pair[r, pi, :, j * P:(j + 1) * P] = oh
                    AT_pair[r, pi, :, j * P:(j + 1) * P] = oh.T
            # keys for this block: gather + mask + transpose (host, bf16)
            km = np.zeros((P, D), np.float32)
            idxs = src[base:base + nseg]
            km[:nseg] = ast[idxs] * written[base:base + nseg, None]
            keysT[r, b] = np.ascontiguousarray(
                km.T.reshape(2, P, P).transpose(1, 0, 2).reshape(P, D)).astype(BF)

    to_bf = lambda a: np.ascontiguousarray(a).astype(BF)
    wkv = np.concatenate([Wk, Wv], axis=1)
    wkv_b = np.stack([to_bf(wkv[0:P]), to_bf(wkv[P:2 * P])])
    wq_b = np.stack([to_bf(Wq[0:P]), to_bf(Wq[P:2 * P])])
    wo_b = np.stack([to_bf(Wo[0:P]), to_bf(Wo[P:2 * P])])
    has_bq = bool(np.any(bq))
    has_bo = bool(np.any(bo))

    nc = _build(nblk, has_bq, has_bo)

    in_maps = []
    for r in range(NCORES):
        in_maps.append({
            "xrb": xrb[r],
            "Ah": A_pair[r],
            "ATh": AT_pair[r],
            "keysT": keysT[r],
            "wkv": wkv_b,
            "wq": wq_b,
            "wo": wo_b,
            "bq": to_bf(bq[None, :]),
            "bo": to_bf(bo[None, :]),
        })

    global _last_in_maps
    _last_in_maps = in_maps
    res = run_bass_kernel_spmd(nc, in_maps, core_ids=list(range(NCORES)))

    out_full = np.zeros((C, OUT_D), np.float32)
    for r in range(NCORES):
        o = res.results[r]["out"]
        for b, (base, nseg) in enumerate(cores[r]):
            if nseg > 0:
                out_full[base:base + nseg] = o[b * P:b * P + nseg]
    return out_full


# revision 13
# speedup vs baseline: 1.5751x; 1.2433x over previous
"""Trainium2 Bass kernel for nn_CFGSubASTExpressionCombiner.

Segment-softmax attention pooling over ragged groups:
  attn_keys = scatter(ast[pdg_val]) by pdg_key (last-write-wins)
  x = ast[map_key]  [M, D]
  per CFG node c: softmax-weighted pooling of v = x@Wv rows whose seg == c,
  with per-head scores k.q (q from attn_keys), then @ Wo + bo.

Strategy: host sorts mapping entries by segment id and assigns each of the 8
cores a contiguous range of segments (~M/8 entries each) -> fully independent
cores, no collectives. Each core processes blocks of <=128 segments
(<=2048 entries = 16 tiles of 128, grouped in 8 pairs).

Host prep (indexing/layout only): the gather x = ast[map_key] and the
per-tile transpose to matmul-lhsT layout are done host-side (bf16), staging a
per-core contiguous stream -- the device does no indirect DMA and no input
transposes. Per-block attention keys are also host-gathered/masked/transposed.

Device per pair (2 tiles of 128 entries, all bf16 matmuls):
  4 mm: kv = xT^T @ [Wk|Wv] into one PSUM tile [128, 1024]
  2 mm: qg = AT^T @ q (gathers per-entry q rows) into [128, 512] PSUM
  1 scalar copy qg -> SBUF; 1 DVE mult k*qg -> scr; 1 DVE reduce -> scores;
  1 scalar exp -> e; 1 gpsimd cast e into rhs2; 1 DVE mult e*v -> rhs2;
  2 mm scatter: nd += A^T @ [e*v | e] accumulated in PSUM over the block.
Per block: q = keysT^T @ Wq, pooled = num/denom, out = pooled @ Wo (+bo).

Scores skip the segment-max subtraction; bounded scores for this problem's
scale make it mathematically identical in f32. Verified ~5e-3 max-rel.
"""
import sys

sys.path.insert(0, "/opt/trn_rl_repo")

from contextlib import ExitStack

import ml_dtypes
import numpy as np

import concourse.bass as bass
import concourse.tile as tile
from concourse import bacc, mybir
from concourse.bass_utils import run_bass_kernel_spmd

P = 128
D = 256
H = 4
DH = 64
OUT_D = 256
NCORES = 8
TPB = 16          # tiles per block (8 pairs)
EPB = TPB * P     # entries per block capacity
bf16 = mybir.dt.bfloat16
f32 = mybir.dt.float32
BF = ml_dtypes.bfloat16

_nc_cache = {}


def _host_blocks(map_key, seg, C):
    """Sort entries by segment, split segments across cores, pack blocks."""
    M = seg.shape[0]
    order = np.argsort(seg, kind="stable")
    seg_s = seg[order].astype(np.int64)
    gid_s = map_key[order].astype(np.int64)
    counts = np.bincount(seg_s, minlength=C)
    cum = np.concatenate([[0], np.cumsum(counts)])

    bounds = [0]
    for r in range(1, NCORES):
        c = int(np.searchsorted(cum, M * r / NCORES))
        bounds.append(max(bounds[-1], min(c, C)))
    bounds.append(C)

    cores = []
    for r in range(NCORES):
        c0, c1 = bounds[r], bounds[r + 1]
        blocks = []
        c = c0
        while c < c1:
            nseg, nent = 0, 0
            while c + nseg < c1 and nseg < P:
                cnt = int(counts[c + nseg])
                if nent + cnt > EPB and nseg > 0:
                    break
                assert cnt <= EPB
                nent += cnt
                nseg += 1
            blocks.append((c, nseg))
            c += nseg
        cores.append(blocks)
    nblk = max(len(b) for b in cores)
    return cores, nblk, cum, gid_s, seg_s


def _build(nblk, has_bq, has_bo):
    key = (nblk, has_bq, has_bo)
    if key in _nc_cache:
        return _nc_cache[key]
    npair = nblk * (TPB // 2)
    nc = bacc.Bacc("TRN2", target_bir_lowering=False, debug=False,
                   num_devices=NCORES)

    xrb_d = nc.dram_tensor("xrb", [npair, P, 2 * D], bf16, kind="ExternalInput").ap()
    A_d = nc.dram_tensor("Ah", [npair, P, 2 * P], bf16, kind="ExternalInput").ap()
    AT_d = nc.dram_tensor("ATh", [npair, P, 2 * P], bf16, kind="ExternalInput").ap()
    keysT_d = nc.dram_tensor("keysT", [nblk, P, D], bf16, kind="ExternalInput").ap()
    wkv_d = nc.dram_tensor("wkv", [2, P, 2 * D], bf16, kind="ExternalInput").ap()
    wq_d = nc.dram_tensor("wq", [2, P, D], bf16, kind="ExternalInput").ap()
    wo_d = nc.dram_tensor("wo", [2, P, OUT_D], bf16, kind="ExternalInput").ap()
    bq_d = nc.dram_tensor("bq", [1, D], bf16, kind="ExternalInput").ap()
    bo_d = nc.dram_tensor("bo", [1, OUT_D], bf16, kind="ExternalInput").ap()
    out_d = nc.dram_tensor("out", [nblk * P, OUT_D], f32, kind="ExternalOutput").ap()

    with tile.TileContext(nc) as tc:
        with ExitStack() as ctx:
            cp = ctx.enter_context(tc.tile_pool(name="const", bufs=1))
            xp = ctx.enter_context(tc.tile_pool(name="xp", bufs=6))
            ap_ = ctx.enter_context(tc.tile_pool(name="ap", bufs=6))
            sp = ctx.enter_context(tc.tile_pool(name="sp", bufs=6))
            qp = ctx.enter_context(tc.tile_pool(name="qp", bufs=3))
            bp = ctx.enter_context(tc.tile_pool(name="bp", bufs=4))
            kv_pool = ctx.enter_context(tc.tile_pool(name="kvp", bufs=2, space="PSUM"))
            qg_pool = ctx.enter_context(tc.tile_pool(name="qgp", bufs=2, space="PSUM"))
            nd_pool = ctx.enter_context(tc.tile_pool(name="ndp", bufs=2, space="PSUM"))

            from concourse.masks import make_identity
            ident = cp.tile([P, P], bf16)
            make_identity(nc, ident[:])
            ones1 = cp.tile([1, P], bf16)
            nc.gpsimd.memset(ones1[:], 1.0)
            wkv0 = cp.tile([P, 2 * D], bf16)
            nc.sync.dma_start(out=wkv0[:], in_=wkv_d[0])
            wkv1 = cp.tile([P, 2 * D], bf16)
            nc.sync.dma_start(out=wkv1[:], in_=wkv_d[1])
            wq0 = cp.tile([P, D], bf16)
            nc.sync.dma_start(out=wq0[:], in_=wq_d[0])
            wq1 = cp.tile([P, D], bf16)
            nc.sync.dma_start(out=wq1[:], in_=wq_d[1])
            wo0 = cp.tile([P, OUT_D], bf16)
            nc.sync.dma_start(out=wo0[:], in_=wo_d[0])
            wo1 = cp.tile([P, OUT_D], bf16)
            nc.sync.dma_start(out=wo1[:], in_=wo_d[1])
            bq_r = cp.tile([1, D], bf16)
            nc.sync.dma_start(out=bq_r[:], in_=bq_d[:, :])
            bo_r = cp.tile([1, OUT_D], bf16)
            nc.sync.dma_start(out=bo_r[:], in_=bo_d[:, :])

            # ---- q computation for one block (emitted staggered) ----
            q_tiles = {}

            def emit_q_setup(b):
                keysT = qp.tile([P, D], bf16, tag="keysT")
                nc.sync.dma_start(out=keysT[:], in_=keysT_d[b])
                q_ps = qg_pool.tile([P, D], f32, tag="qg")
                nc.tensor.matmul(out=q_ps[:], lhsT=keysT[:, 0:P], rhs=wq0[:],
                                 start=True, stop=False)
                nc.tensor.matmul(out=q_ps[:], lhsT=keysT[:, P:D], rhs=wq1[:],
                                 start=False, stop=not has_bq)
                if has_bq:
                    nc.tensor.matmul(out=q_ps[:], lhsT=ones1[:], rhs=bq_r[:],
                                     start=False, stop=True)
                q_sb = qp.tile([P, D], bf16, tag="qsb")
                nc.scalar.copy(out=q_sb[:], in_=q_ps[:])
                q_tiles[b] = q_sb

            emit_q_setup(0)
            if nblk > 1:
                emit_q_setup(1)
            for b in range(nblk):
                if b + 2 < nblk:
                    emit_q_setup(b + 2)
                q_sb = q_tiles.pop(b)
                nd_ps = nd_pool.tile([P, D + H], f32, tag="ndp")

                # software pipeline: stage A (kv/qg matmuls + scores) for pair
                # pr, then stage B (e*v + scatter) for pair pr-1, so the PE
                # never waits on the DVE/scalar chain of the current pair.
                pend = None

                def stage_b(st):
                    pr, A2, rhs2, rhs4, kv4, e2f = st
                    nc.vector.tensor_tensor(
                        out=rhs4[:, :, 0:D].rearrange("p j (h x) -> p j h x",
                                                      x=DH),
                        in0=kv4[:, :, 1, :, :],
                        in1=rhs4[:, :, D:D + H, None].to_broadcast([P, 2, H, DH]),
                        op=mybir.AluOpType.mult)
                    for j in range(2):
                        nc.tensor.matmul(
                            out=nd_ps[:, 0:D + H],
                            lhsT=A2[:, j * P:(j + 1) * P],
                            rhs=rhs2[:, j * (D + H):(j + 1) * (D + H)],
                            start=(pr == 0 and j == 0),
                            stop=(pr == TPB // 2 - 1 and j == 1))

                for pr in range(TPB // 2):
                    pi = b * (TPB // 2) + pr
                    xr2 = xp.tile([P, 2 * D], bf16)
                    nc.sync.dma_start(out=xr2[:], in_=xrb_d[pi])
                    A2 = ap_.tile([P, 2 * P], bf16, tag="A")
                    nc.sync.dma_start(out=A2[:], in_=A_d[pi])
                    AT2 = ap_.tile([P, 2 * P], bf16, tag="AT")
                    nc.sync.dma_start(out=AT2[:], in_=AT_d[pi])
                    rhs2 = sp.tile([P, 2 * (D + H)], bf16, tag="rhs2")
                    sc2 = sp.tile([P, 2 * H], f32, tag="sc2")
                    e2f = sp.tile([P, 2 * H], f32, tag="e2f")
                    scr = sp.tile([P, 2 * D], bf16, tag="scr")
                    kv2_ps = kv_pool.tile([P, 4 * D], f32, tag="kvp")
                    qg2_ps = qg_pool.tile([P, 2 * D], f32, tag="qg")
                    for j in range(2):
                        xT = xr2[:, j * D:(j + 1) * D]
                        nc.tensor.matmul(out=kv2_ps[:, j * 2 * D:(j + 1) * 2 * D],
                                         lhsT=xT[:, 0:P], rhs=wkv0[:],
                                         start=True, stop=False)
                        nc.tensor.matmul(out=kv2_ps[:, j * 2 * D:(j + 1) * 2 * D],
                                         lhsT=xT[:, P:D], rhs=wkv1[:],
                                         start=False, stop=True)
                        nc.tensor.matmul(out=qg2_ps[:, j * D:(j + 1) * D],
                                         lhsT=AT2[:, j * P:(j + 1) * P], rhs=q_sb[:],
                                         start=True, stop=True)
                    qg2_sb = sp.tile([P, 2 * D], bf16, tag="qg_sb")
                    nc.scalar.copy(out=qg2_sb[:], in_=qg2_ps[:])
                    # scores: k * qg for both tiles in one op, then one reduce
                    kv4 = kv2_ps[:].rearrange("p (j t h x) -> p j t h x",
                                              j=2, t=2, h=H)
                    nc.vector.tensor_tensor(
                        out=scr[:].rearrange("p (j h x) -> p j h x", j=2, h=H),
                        in0=kv4[:, :, 0, :, :],
                        in1=qg2_sb[:].rearrange("p (j h x) -> p j h x", j=2, h=H),
                        op=mybir.AluOpType.mult)
                    nc.vector.reduce_sum(
                        out=sc2[:],
                        in_=scr[:].rearrange("p (g x) -> p g x", x=DH),
                        axis=mybir.AxisListType.X)
                    nc.scalar.activation(
                        out=e2f[:], in_=sc2[:],
                        func=mybir.ActivationFunctionType.Exp,
                        scale=float(1.0 / np.sqrt(DH)))
                    rhs4 = rhs2[:].rearrange("p (j q) -> p j q", j=2)
                    nc.gpsimd.tensor_scalar(
                        out=rhs4[:, :, D:D + H],
                        in0=e2f[:].rearrange("p (j h) -> p j h", j=2),
                        scalar1=1.0, scalar2=None, op0=mybir.AluOpType.mult)
                    if pend is not None:
                        stage_b(pend)
                    pend = (pr, A2, rhs2, rhs4, kv4, e2f)
                stage_b(pend)

                # ---- block finish: pooled = num/denom, out = pooled@Wo+bo ----
                dsb = bp.tile([P, H], f32)
                nc.vector.tensor_scalar(out=dsb[:], in0=nd_ps[:, D:D + H],
                                        scalar1=1e-9, scalar2=None,
                                        op0=mybir.AluOpType.add)
                recip = bp.tile([P, H], f32)
                nc.vector.reciprocal(out=recip[:], in_=dsb[:])
                pooled = bp.tile([P, D], bf16)
                nc.vector.tensor_tensor(
                    out=pooled[:, :].rearrange("p (h x) -> p h x", x=DH),
                    in0=nd_ps[:, 0:D].rearrange("p (h x) -> p h x", x=DH),
                    in1=recip[:, :, None].to_broadcast([P, H, DH]),
                    op=mybir.AluOpType.mult)
                pooledT_ps = qg_pool.tile([P, D], bf16, tag="qg")
                nc.tensor.transpose(out=pooledT_ps[:, 0:P], in_=pooled[:, 0:P],
                                    identity=ident[:])
                nc.tensor.transpose(out=pooledT_ps[:, P:D], in_=pooled[:, P:D],
                                    identity=ident[:])
                pooledT = bp.tile([P, D], bf16)
                nc.scalar.copy(out=pooledT[:], in_=pooledT_ps[:, 0:D])
                o_ps = qg_pool.tile([P, OUT_D], f32, tag="qg")
                nc.tensor.matmul(out=o_ps[:], lhsT=pooledT[:, 0:P], rhs=wo0[:],
                                 start=True, stop=False)
                nc.tensor.matmul(out=o_ps[:], lhsT=pooledT[:, P:D], rhs=wo1[:],
                                 start=False, stop=not has_bo)
                if has_bo:
                    nc.tensor.matmul(out=o_ps[:], lhsT=ones1[:], rhs=bo_r[:],
                                     start=False, stop=True)
                out_sb = bp.tile([P, OUT_D], f32)
                nc.scalar.copy(out=out_sb[:], in_=o_ps[:])
                nc.sync.dma_start(out=out_d[b * P:(b + 1) * P, :], in_=out_sb[:])

    nc.compile()
    _nc_cache[key] = nc
    return nc


def kernel(**inputs):
    ast = np.ascontiguousarray(np.asarray(inputs["ast_nodes_encodings"], np.float32))
    map_key = np.asarray(inputs["ast_node_idx_to_pdg_node_idx_mapping_key"]).astype(np.int64)
    seg = np.asarray(inputs["ast_node_idx_to_pdg_node_idx_mapping_value"]).astype(np.int64)
    pdg_key = np.asarray(inputs["pdg_node_idx_to_sub_ast_root_idx_mapping_key"]).astype(np.int64)
    pdg_val = np.asarray(inputs["pdg_node_idx_to_sub_ast_root_idx_mapping_value"]).astype(np.int64)
    C = int(np.asarray(inputs["nr_cfg_nodes"]))
    Wq = np.asarray(inputs["Wq"], np.float32)
    bq = np.asarray(inputs["bq"], np.float32)
    Wk = np.asarray(inputs["Wk"], np.float32)
    bk = np.asarray(inputs["bk"], np.float32)
    Wv = np.asarray(inputs["Wv"], np.float32)
    bv = np.asarray(inputs["bv"], np.float32)
    Wo = np.asarray(inputs["Wo"], np.float32)
    bo = np.asarray(inputs["bo"], np.float32)
    assert not (np.any(bk) or np.any(bv)), "nonzero bk/bv not staged"

    # attn_keys source resolution: last-write-wins scatter -> gather + mask
    src = np.zeros(C, np.int64)
    src[pdg_key] = pdg_val
    written = np.zeros(C, bool)
    written[pdg_key] = True

    cores, nblk, cum, gid_s, seg_s = _host_blocks(map_key, seg, C)
    npair = nblk * (TPB // 2)

    ast_bf = ast.astype(BF)

    xrb = np.zeros((NCORES, npair, P, 2 * D), BF)
    A_pair = np.zeros((NCORES, npair, P, 2 * P), BF)
    AT_pair = np.zeros((NCORES, npair, P, 2 * P), BF)
    keysT = np.zeros((NCORES, nblk, P, D), BF)
    iota = np.arange(P)
    for r in range(NCORES):
        for b, (base, nseg) in enumerate(cores[r]):
            s, e = cum[base], cum[base + nseg]
            n = e - s
            g = np.zeros(EPB, np.int64)
            sl = np.full(EPB, -1, np.int64)
            g[:n] = gid_s[s:e]
            sl[:n] = seg_s[s:e] - base
            gt = g.reshape(TPB, P)
            st = sl.reshape(TPB, P)
            # [TPB, 128, 256] lhsT layout: xT[t, p, c*128+m] = x[t, m, c*128+p]
            xT = ast_bf[gt].transpose(0, 2, 1).reshape(TPB, 2, P, P).transpose(
                0, 2, 1, 3).reshape(TPB, P, D)
            onehot = (st[:, :, None] == iota[None, None, :])
            for pr in range(TPB // 2):
                pi = b * (TPB // 2) + pr
                xrb[r, pi] = np.concatenate([xT[2 * pr], xT[2 * pr + 1]], axis=1)
                for j in range(2):
                    oh = onehot[pr * 2 + j]
                    A_pair[r, pi, :, j * P:(j + 1) * P] = oh
                    AT_pair[r, pi, :, j * P:(j + 1) * P] = oh.T
            # keys for this block: gather + mask + transpose (host, bf16)
            km = np.zeros((P, D), np.float32)
            idxs = src[base:base + nseg]
            km[:nseg] = ast[idxs] * written[base:base + nseg, None]
            keysT[r, b] = np.ascontiguousarray(
                km.T.reshape(2, P, P).transpose(1, 0, 2).reshape(P, D)).astype(BF)

    to_bf = lambda a: np.ascontiguousarray(a).astype(BF)
    wkv = np.concatenate([Wk, Wv], axis=1)
    wkv_b = np.stack([to_bf(wkv[0:P]), to_bf(wkv[P:2 * P])])
    wq_b = np.stack([to_bf(Wq[0:P]), to_bf(Wq[P:2 * P])])
    wo_b = np.stack([to_bf(Wo[0:P]), to_bf(Wo[P:2 * P])])
    has_bq = bool(np.any(bq))
    has_bo = bool(np.any(bo))

    nc = _build(nblk, has_bq, has_bo)

    in_maps = []
    for r in range(NCORES):
        in_maps.append({
            "xrb": xrb[r],
            "Ah": A_pair[r],
            "ATh": AT_pair[r],
            "keysT": keysT[r],
            "wkv": wkv_b,
            "wq": wq_b,
            "wo": wo_b,
            "bq": to_bf(bq[None, :]),
            "bo": to_bf(bo[None, :]),
        })

    global _last_in_maps
    _last_in_maps = in_maps
    res = run_bass_kernel_spmd(nc, in_maps, core_ids=list(range(NCORES)))

    out_full = np.zeros((C, OUT_D), np.float32)
    for r in range(NCORES):
        o = res.results[r]["out"]
        for b, (base, nseg) in enumerate(cores[r]):
            if nseg > 0:
                out_full[base:base + nseg] = o[b * P:b * P + nseg]
    return out_full
